# revision 3
# baseline (speedup 1.0000x reference)
"""GRU-with-skip Trainium2 kernel, v7.

v2 -> v3 (driven by trace analysis of v2 @ 10.4ms):
- HAM warmth: phase-1/3 work is emitted in ~1us pieces BETWEEN a step's gate
  burst and its serial tail, so the PE instruction stream (executed in order)
  has fill work during the tail and the clock gate stays at 8/8.
- Gate chains reordered (ko, q, g): 3 consecutive matmuls share one stationary
  operand, giving walrus the chance to skip redundant LDWEIGHTS.
- Shorter tail: h_new = zp*n + z*h with the z*h branch computed during the
  gate burst; tanh-side ops split into column halves so transposes/casts of
  half 0 unblock the next step early; hT casts moved to ScalarE.
- Phase-1 staging writes repacked (evq) -> 7 DMAs/tile instead of 15.
"""

import sys

for _p in ("/opt/trn_rl_repo", "/root/.axon_site/_ro/trn_rl_repo"):
    if _p not in sys.path:
        sys.path.insert(0, _p)

import numpy as np
import ml_dtypes

import concourse.bass as bass
import concourse.tile as tile
from concourse import bacc, mybir
from concourse.bass_utils import run_bass_kernel_spmd

F32 = mybir.dt.float32
BF16 = mybir.dt.bfloat16
AF = mybir.ActivationFunctionType
ALU = mybir.AluOpType

P = 128
B, T, I, H, O = 128, 1024, 512, 1024, 512
NCORES = 8
BC = B // NCORES
NQ = 4
QH = H // NQ  # 256
HF = QH // 2  # 128 (half of a quarter-stack column block)
LN_EPS = 1e-5


def build_nc(t_steps: int = T):
    nc = bacc.Bacc(None, target_bir_lowering=False)
    TB = min(P, t_steps)
    nb = t_steps // TB

    x_in = nc.dram_tensor("x", [BC, t_steps, I], F32, kind="ExternalInput")
    wiT = nc.dram_tensor("wiT", [I, 4 * H], BF16, kind="ExternalInput")
    biasi1 = nc.dram_tensor("biasi1", [1, 4 * H], BF16, kind="ExternalInput")
    whT = nc.dram_tensor("whT", [H, 3 * H], BF16, kind="ExternalInput")
    woT = nc.dram_tensor("woT", [H, O], BF16, kind="ExternalInput")
    bo1 = nc.dram_tensor("bo1", [1, O], BF16, kind="ExternalInput")
    ones1 = nc.dram_tensor("ones1", [1, P], BF16, kind="ExternalInput")
    identF = nc.dram_tensor("identF", [P, P], F32, kind="ExternalInput")
    identB = nc.dram_tensor("identB", [P, P], BF16, kind="ExternalInput")
    bhn_st = nc.dram_tensor("bhn_st", [P, QH], BF16, kind="ExternalInput")
    i16x4 = nc.dram_tensor("i16x4", [P, BC], BF16, kind="ExternalInput")
    out = nc.dram_tensor("out", [BC, t_steps, O], F32, kind="ExternalOutput")

    with tile.TileContext(nc) as tc:
        with (
            tc.tile_pool(name="dram", bufs=1, space="DRAM") as dram,
            tc.tile_pool(name="const", bufs=1) as const,
            tc.tile_pool(name="p1s", bufs=2) as p1s,
            tc.tile_pool(name="p1q", bufs=2) as p1q,
            tc.tile_pool(name="p1e", bufs=3) as p1e,
            tc.tile_pool(name="p2s", bufs=6) as p2s,
            tc.tile_pool(name="p2t", bufs=2) as p2t,
            tc.tile_pool(name="p3s", bufs=2) as p3s,
            tc.tile_pool(name="p3t", bufs=2) as p3t,
            tc.tile_pool(name="gps", bufs=1, space="PSUM") as gps,
            tc.tile_pool(name="tpsum", bufs=3, space="PSUM") as tpsum,
            tc.tile_pool(name="mmps", bufs=3, space="PSUM") as mmps,
        ):
            st2 = dram.tile([t_steps, P, 3 * QH], BF16)
            hs_st = dram.tile([t_steps, P, QH], F32)
            skb = dram.tile([BC, t_steps, H], BF16)

            wiT_sb = const.tile([P, I // P, 4 * H], BF16)
            nc.sync.dma_start(wiT_sb, wiT[:].rearrange("(ko p) m -> p ko m", p=P))
            whT_sb = const.tile([P, H // P, 3 * H], BF16)
            nc.sync.dma_start(whT_sb, whT[:].rearrange("(ko p) m -> p ko m", p=P))
            woT_sb = const.tile([P, H // P, O], BF16)
            nc.sync.dma_start(woT_sb, woT[:].rearrange("(ko p) m -> p ko m", p=P))
            biasi1_sb = const.tile([1, 4 * H], BF16)
            nc.sync.dma_start(biasi1_sb, biasi1[:])
            bo1_sb = const.tile([1, O], BF16)
            nc.sync.dma_start(bo1_sb, bo1[:])
            ones1_sb = const.tile([1, P], BF16)
            nc.sync.dma_start(ones1_sb, ones1[:])
            identF_sb = const.tile([P, P], F32)
            nc.sync.dma_start(identF_sb, identF[:])
            identB_sb = const.tile([P, P], BF16)
            nc.sync.dma_start(identB_sb, identB[:])
            bhn_st_sb = const.tile([P, QH], BF16)
            nc.sync.dma_start(bhn_st_sb, bhn_st[:])
            i16x4_sb = const.tile([P, BC], BF16)
            nc.sync.dma_start(i16x4_sb, i16x4[:])
            eps_sb = const.tile([P, 1], F32)
            nc.vector.memset(eps_sb, LN_EPS)

            # ---------------- phase 1 (piecewise) ----------------
            # state per in-flight tile: dict with xt/xT/evq
            def p1_start(stt, b, tb):
                t0 = tb * TB
                stt.update(b=b, tb=tb, t0=t0)
                stt["xt"] = p1s.tile([TB, I], F32, tag="xt", name="xt")
                nc.sync.dma_start(stt["xt"], x_in[b, t0 : t0 + TB, :])
                stt["evq"] = p1q.tile([TB, NQ, 3 * QH], BF16, tag="evq", name="evq")

            def p1_piece_mm(stt, j):
                b, t0 = stt["b"], stt["t0"]
                if j == 0:
                    px = tpsum.tile([P, I // P, TB], F32, tag="tp")
                    for jj in range(I // P):
                        nc.tensor.transpose(
                            px[:, jj],
                            stt["xt"][:, jj * P : (jj + 1) * P],
                            identF_sb[:TB, :TB],
                        )
                    xT = p1s.tile([P, I // P, TB], BF16, tag="xT")
                    nc.scalar.copy(xT, px)
                    stt["xT"] = xT
                g, hb = j // 2, j % 2
                pm = mmps.tile([TB, 512], F32, tag="pm")
                nc.tensor.matmul(
                    pm,
                    ones1_sb[:, :TB],
                    biasi1_sb[:, j * 512 : (j + 1) * 512],
                    start=True,
                    stop=False,
                    skip_group_check=True,
                )
                for ko in range(I // P):
                    nc.tensor.matmul(
                        pm,
                        stt["xT"][:, ko],
                        wiT_sb[:, ko, j * 512 : (j + 1) * 512],
                        start=False,
                        stop=(ko == I // P - 1),
                        skip_group_check=True,
                    )
                stt["pm"] = pm

            def p1_piece_evac(stt, j):
                b, t0 = stt["b"], stt["t0"]
                g, hb = j // 2, j % 2
                pm = stt.pop("pm")
                if g < 3:
                    # pack the two quarter-halves into the staging tile
                    for h2 in range(2):
                        q = 2 * hb + h2
                        dst = stt["evq"][:, q, g * QH : (g + 1) * QH]
                        src = pm[:, h2 * QH : (h2 + 1) * QH]
                        nc.scalar.copy(dst, src)
                    if j == 5:  # r,z,n all packed -> 4 staging DMAs
                        for q in range(NQ):
                            nc.gpsimd.dma_start(
                                st2[t0 : t0 + TB, 32 * q + stt["b"], :],
                                stt["evq"][:, q, :],
                            )
                else:
                    ev = p1e.tile([TB, 512], BF16, tag="ev")
                    if hb == 0:
                        nc.vector.tensor_copy(ev, pm)
                    else:
                        nc.scalar.copy(ev, pm)
                    nc.gpsimd.dma_start(
                        skb[b, t0 : t0 + TB, hb * 512 : (hb + 1) * 512], ev
                    )

            def p1_whole(b, tb):
                stt = {}
                p1_start(stt, b, tb)
                for j in range(8):
                    p1_piece_mm(stt, j)
                    p1_piece_evac(stt, j)

            # ---------------- phase 3 (two pieces) ----------------
            def p3_a(stt, b, tb):
                t0 = tb * TB
                stt.update(b=b, tb=tb, t0=t0)
                hs_t = p3s.tile([TB, NQ, QH], F32, tag="hst")
                nc.sync.dma_start(
                    hs_t,
                    hs_st[t0 : t0 + TB]
                    .rearrange("t (q r) c -> t q r c", q=NQ)[:, :, b, :],
                )
                sk_t = p3s.tile([TB, H], BF16, tag="skt")
                nc.sync.dma_start(sk_t, skb[b, t0 : t0 + TB, :])
                comb = p3t.tile([TB, H], BF16, tag="comb")
                hsf = hs_t.rearrange("t q c -> t (q c)")
                for cc in range(4):
                    cs = slice(cc * QH, (cc + 1) * QH)
                    nc.vector.tensor_add(comb[:, cs], hsf[:, cs], sk_t[:, cs])
                st = p3t.tile([TB, 2, 6], F32, tag="st")
                nc.vector.bn_stats(st[:, 0], comb[:, :512])
                nc.vector.bn_stats(st[:, 1], comb[:, 512:])
                mv = p3t.tile([TB, 2], F32, tag="mv")
                nc.vector.bn_aggr(mv, st)
                rstd = p3t.tile([TB, 1], F32, tag="rstd")
                nc.scalar.activation(rstd, mv[:, 1:2], AF.Sqrt, bias=eps_sb[:TB])
                nc.vector.reciprocal(rstd, rstd)
                nbias = p3t.tile([TB, 1], F32, tag="nbias")
                nc.vector.scalar_tensor_tensor(
                    out=nbias,
                    in0=mv[:, 0:1],
                    scalar=-1.0,
                    in1=rstd,
                    op0=ALU.mult,
                    op1=ALU.mult,
                )
                nrm = p3t.tile([TB, H], BF16, tag="nrm")
                for cc in range(2):
                    cs = slice(cc * 512, (cc + 1) * 512)
                    nc.scalar.activation(
                        nrm[:, cs], comb[:, cs], AF.Identity, bias=nbias, scale=rstd
                    )
                stt["nrm"] = nrm

            def p3_b(stt):
                b, t0 = stt["b"], stt["t0"]
                nrm = stt["nrm"]
                nT = p3t.tile([P, H // P, TB], BF16, tag="nT")
                for j2 in range(2):
                    ntp = tpsum.tile([P, 4, TB], BF16, tag="tp")
                    for j in range(4):
                        jj = j2 * 4 + j
                        nc.tensor.transpose(
                            ntp[:, j],
                            nrm[:, jj * P : (jj + 1) * P],
                            identB_sb[:TB, :TB],
                        )
                    nc.vector.tensor_copy(nT[:, j2 * 4 : j2 * 4 + 4], ntp)
                po = mmps.tile([TB, O], F32, tag="pm")
                nc.tensor.matmul(
                    po, ones1_sb[:, :TB], bo1_sb, start=True, stop=False,
                    skip_group_check=True,
                )
                for ko in range(H // P):
                    nc.tensor.matmul(
                        po,
                        nT[:, ko],
                        woT_sb[:, ko],
                        start=False,
                        stop=(ko == H // P - 1),
                        skip_group_check=True,
                    )
                o_sb = p3t.tile([TB, O], F32, tag="o")
                nc.scalar.copy(o_sb, po)
                nc.scalar.dma_start(out[b, t0 : t0 + TB, :], o_sb)

            # ---------------- schedules ----------------
            piecewise = nb >= 3
            if piecewise:
                for b in range(BC):
                    p1_whole(b, 0)
                p1_whole(0, 1)  # tile (0, 1): its pieces fall before t=0
                p1_states = {}
            else:
                for tb in range(nb):
                    for b in range(BC):
                        p1_whole(b, tb)

            # recurrence state
            h_st = p2t.tile([P, QH], F32, tag="h")
            nc.vector.memset(h_st, 0.0)
            hT = p2t.tile([P, 2, P], BF16, tag="hT")
            nc.vector.memset(hT, 0.0)
            P_rz = gps.tile([P, 2, QH], F32, tag="Prz")
            P_n = gps.tile([P, QH], F32, tag="Pn")
            nc.vector.memset(P_rz, 0.0)
            nc.vector.memset(P_n, 0.0)

            KO = H // P
            whT_rz = whT_sb[:, :, : 2 * H].rearrange("p k (g c) -> p k g c", g=2)

            p3_states = {}

            def emit_injects(gi):
                for q in range(NQ):
                    sl = slice(32 * q, 32 * q + BC)
                    nc.tensor.matmul(
                        P_rz[sl],
                        i16x4_sb[sl],
                        gi[sl, : 2 * QH].rearrange("p (g c) -> p g c", g=2),
                        start=True,
                        stop=False,
                        skip_group_check=True,
                        tile_position=(32 * q, 32 * q),
                    )
                for q in range(NQ):
                    sl = slice(32 * q, 32 * q + BC)
                    nc.tensor.matmul(
                        P_n[sl],
                        i16x4_sb[sl],
                        bhn_st_sb[sl],
                        start=True,
                        stop=False,
                        skip_group_check=True,
                        tile_position=(32 * q, 32 * q),
                    )

            gis = {}
            for tt in range(min(2, t_steps)):
                gis[tt] = p2s.tile([P, 3 * QH], BF16, tag="gi", name="gi")
                nc.sync.dma_start(gis[tt], st2[tt])
            emit_injects(gis[0])

            for t in range(t_steps):
                gi = gis.pop(t)
                # --- gate burst: combined [r|z] 2-plane chains, then n ---
                # (injects for step t were emitted at the end of step t-1)
                for ki, ko in enumerate((0, 2, 4, 6, 1, 3, 5, 7)):
                    lhs = hT[:, ko % 2, 32 * (ko // 2) : 32 * (ko // 2) + BC]
                    for q in range(NQ):
                        sl = slice(32 * q, 32 * q + BC)
                        nc.tensor.matmul(
                            P_rz[sl],
                            lhs,
                            whT_rz[:, ko, :, q * QH : (q + 1) * QH],
                            start=False,
                            stop=(ki == KO - 1),
                            skip_group_check=True,
                            tile_position=(0, 32 * q),
                        )
                for ki, ko in enumerate((0, 2, 4, 6, 1, 3, 5, 7)):
                    lhs = hT[:, ko % 2, 32 * (ko // 2) : 32 * (ko // 2) + BC]
                    for q in range(NQ):
                        sl = slice(32 * q, 32 * q + BC)
                        nc.tensor.matmul(
                            P_n[sl],
                            lhs,
                            whT_sb[:, ko, 2 * H + q * QH : 2 * H + (q + 1) * QH],
                            start=False,
                            stop=(ki == KO - 1),
                            skip_group_check=True,
                            tile_position=(0, 32 * q),
                        )

                # --- phase fill: matmul halves go right after the chains so
                # the in-order PE stream has work during the tail; the
                # ACT/DVE evac halves are deferred to after the tail ops ---
                p1st = p3st_a = p3st_b = None
                if piecewise:
                    k = t // TB
                    # ph1 runs 1 block ahead, shifted 8 steps early
                    p = t + 8
                    pk = p // TB + 1
                    rel_p = p % TB
                    tile_i_p = rel_p // 8
                    jp = rel_p % 8
                    if 1 <= pk < nb:
                        key = (pk, tile_i_p)
                        if jp == 0:
                            p1_states[key] = {}
                            p1_start(p1_states[key], tile_i_p, pk)
                        p1st = (p1_states[key], jp)
                        p1_piece_mm(*p1st)
                        if jp == 7:
                            del p1_states[key]
                    if k >= 1:
                        rel = t % TB
                        tile_i = rel // 8
                        if rel % 8 == 1:
                            p3_states[(k - 1, tile_i)] = {}
                            p3st_a = p3_states[(k - 1, tile_i)]
                        elif rel % 8 == 5:
                            p3st_b = p3_states.pop((k - 1, tile_i))
                            p3_b(p3st_b)

                # --- tail ---
                r_sb = p2t.tile([P, QH], BF16, tag="r")
                nc.scalar.activation(r_sb[:, :HF], P_rz[:, 0, :HF], AF.Sigmoid)
                nc.scalar.activation(r_sb[:, HF:], P_rz[:, 0, HF:], AF.Sigmoid)
                zp_sb = p2t.tile([P, QH], BF16, tag="zp")
                nc.scalar.activation(zp_sb, P_rz[:, 1], AF.Sigmoid, scale=-1.0)
                z_sb = p2t.tile([P, QH], BF16, tag="z")
                nc.gpsimd.tensor_scalar(
                    out=z_sb, in0=zp_sb, scalar1=-1.0, scalar2=1.0,
                    op0=ALU.mult, op1=ALU.add,
                )
                bzh = p2t.tile([P, QH], F32, tag="bzh")
                nc.gpsimd.tensor_mul(bzh, z_sb, h_st)  # z*h, off critical path

                h_new = p2t.tile([P, QH], F32, tag="h")
                ptr = tpsum.tile([P, 2, P], F32, tag="tp")
                hT_new = p2t.tile([P, 2, P], BF16, tag="hT")
                t1s, t2s, nss = [], [], []
                for c in range(2):
                    cs = slice(c * HF, (c + 1) * HF)
                    t1 = p2t.tile([P, HF], BF16, tag=f"t1{c}", name="t1")
                    nc.vector.tensor_mul(t1, r_sb[:, cs], P_n[:, cs])
                    t2 = p2t.tile([P, HF], BF16, tag=f"t2{c}", name="t2")
                    nc.vector.tensor_add(t2, t1, gi[:, 2 * QH + c * HF : 2 * QH + (c + 1) * HF])
                    t2s.append(t2)
                for c in range(2):
                    n_sb = p2t.tile([P, HF], BF16, tag=f"n{c}", name="n_sb")
                    nc.scalar.activation(n_sb, t2s[c], AF.Tanh)
                    nss.append(n_sb)
                for c in range(2):
                    cs = slice(c * HF, (c + 1) * HF)
                    m1 = p2t.tile([P, HF], BF16, tag=f"m1{c}", name="m1")
                    nc.vector.tensor_mul(m1, nss[c], zp_sb[:, cs])
                    nc.vector.tensor_add(h_new[:, cs], m1, bzh[:, cs])

                # prefetch gi two steps out; next step's injects are PE fill
                if t + 2 < t_steps:
                    gis[t + 2] = p2s.tile([P, 3 * QH], BF16, tag="gi", name="gi")
                    nc.sync.dma_start(gis[t + 2], st2[t + 2])
                if t + 1 < t_steps:
                    emit_injects(gis[t + 1])

                for c in range(2):
                    nc.tensor.transpose(
                        ptr[:, c], h_new[:, c * P : (c + 1) * P], identF_sb
                    )
                    if c == 0:
                        nc.scalar.copy(hT_new[:, c], ptr[:, c])
                    else:
                        nc.vector.tensor_copy(hT_new[:, c], ptr[:, c])
                nc.gpsimd.dma_start(hs_st[t], h_new)
                h_st, hT = h_new, hT_new

                # deferred phase evacs/DVE work (lower priority than the tail)
                if p1st is not None:
                    p1_piece_evac(*p1st)
                if p3st_a is not None:
                    k = t // TB
                    rel = t % TB
                    p3_a(p3st_a, rel // 8, k - 1)

            # epilogue
            if piecewise:
                for tile_i in range(BC):
                    stt = {}
                    p3_a(stt, tile_i, nb - 1)
                    p3_b(stt)
            else:
                for tb in range(nb):
                    for b in range(BC):
                        stt = {}
                        p3_a(stt, b, tb)
                        p3_b(stt)

    nc.finalize()
    return nc


def prep_host_inputs(inputs):
    g = {k: np.asarray(v, dtype=np.float32) for k, v in inputs.items()}
    bf = ml_dtypes.bfloat16
    wiT = np.concatenate(
        [g["Wir"].T, g["Wiz"].T, g["Win"].T, g["Wskip"].T], axis=1
    )
    biasi1 = np.concatenate(
        [g["bir"] + g["bhr"], g["biz"] + g["bhz"], g["bin_"], g["bskip"]]
    )[None, :]
    whT = np.concatenate([g["Whr"].T, g["Whz"].T, g["Whn"].T], axis=1)
    woT = np.ascontiguousarray((g["Wout"] * g["gamma"][None, :]).T)
    bo1 = (g["bout"] + g["Wout"] @ g["beta"])[None, :]
    ones1 = np.ones((1, P), np.float32)
    identF = np.eye(P, dtype=np.float32)
    i16x4 = np.zeros((P, BC), np.float32)
    for q in range(NQ):
        i16x4[32 * q : 32 * q + BC] = np.eye(BC)
    bhn_st = np.zeros((P, QH), np.float32)
    for q in range(NQ):
        bhn_st[32 * q : 32 * q + BC] = g["bhn"][QH * q : QH * (q + 1)][None, :]
    return dict(
        wiT=np.ascontiguousarray(wiT).astype(bf),
        biasi1=biasi1.astype(bf),
        whT=np.ascontiguousarray(whT).astype(bf),
        woT=woT.astype(bf),
        bo1=bo1.astype(bf),
        ones1=ones1.astype(bf),
        identF=identF,
        identB=identF.astype(bf),
        bhn_st=bhn_st.astype(bf),
        i16x4=i16x4.astype(bf),
    )


_NC_CACHE = {}


def run(inputs, t_steps=T, trace=False):
    if t_steps not in _NC_CACHE:
        _NC_CACHE[t_steps] = build_nc(t_steps)
    nc = _NC_CACHE[t_steps]
    shared = prep_host_inputs(inputs)
    x = np.asarray(inputs["x"], dtype=np.float32)[:, :t_steps]
    in_maps = [
        {"x": np.ascontiguousarray(x[c * BC : (c + 1) * BC]), **shared}
        for c in range(NCORES)
    ]
    res = run_bass_kernel_spmd(
        nc, in_maps, core_ids=list(range(NCORES)), trace=trace
    )
    outp = np.concatenate([res.results[c]["out"] for c in range(NCORES)], axis=0)
    return outp, res


def kernel(**inputs) -> np.ndarray:
    outp, _ = run(inputs)
    return outp
